# revision 1
# baseline (speedup 1.0000x reference)
"""Trainium2 Bass kernel for an attention block with a non-standard
(query-axis) softmax and causal mask.

Math per batch element b (T=2048 tokens, C=K=V=512):
    q = x @ Wq.T + bq ; k = x @ Wk.T + bk ; v = x @ Wv.T + bv
    logits[j, i] = q[j] . k[i]                     (j=query, i=key)
    masked = -inf where i > j
    probs = softmax(masked / sqrt(512), axis=j)    <-- softmax over QUERY axis
    read[j] = sum_i probs[j, i] * v[i]
    out = concat(x, read)                          [T, 1024]

Distribution: pure data-parallel, batch b -> core b (8 batches, 8 cores),
weights replicated, no collectives.

Kernel layout choice: compute L^T[i, j] (key index i on partitions, query
index j on the free dim).  The axis=1 (query-axis) softmax then reduces
along the free dim, which ACT fuses into the exp via accum_out.  The
causal mask in this layout zeroes j < i: only j-chunks at or right of the
diagonal are computed, and the leading fully-masked 128-col strips of the
diagonal chunk are trimmed too.  The softmax normalizer 1/sum is folded
into a rescale of V rows.  P^T[i, j] is exactly the lhsT the read-matmul
needs: read = P^T.T @ V'.

All matmuls run in bf16 (1 cycle/row on the PE vs 4 for fp32); input
transposes and weight pre-scaling (1/sqrt(512) folded into Wq, bq) are done
on the host in numpy.

Scheduling notes (from perfetto trace analysis): everything rides one
HWDGE queue FIFO, so input loads are emitted strictly in first-use order
and the output/passthrough DMAs go on the GPSIMD software-DGE queues
instead.  Full-width warm-up matmuls on a memset tile run during the
initial load so the PE's HAM clock gate is at full rate when real work
arrives (rank-1 warm-ups don't register on the HAM activity monitor).
"""

import math

import numpy as np
import ml_dtypes

P = 128
B, T, C = 8, 2048, 512
NT = T // P     # 16 row tiles
NK = C // P     # 4 contraction / k tiles
NJ = T // 512   # 4 query chunks of 512
NCORES = 8
NEG = -1e30

_BUILT = None


def _build_nc():
    import concourse.mybir as mybir
    import concourse.tile as tile
    from concourse import bacc

    f32 = mybir.dt.float32
    bf16 = mybir.dt.bfloat16
    AF = mybir.ActivationFunctionType

    nc = bacc.Bacc("TRN2", target_bir_lowering=False, debug=False,
                   num_devices=NCORES)

    xt_d = nc.dram_tensor("xt", [C, T], bf16, kind="ExternalInput")
    x_d = nc.dram_tensor("x", [T, C], f32, kind="ExternalInput")
    wqt_d = nc.dram_tensor("wqt", [C, C], bf16, kind="ExternalInput")
    wkt_d = nc.dram_tensor("wkt", [C, C], bf16, kind="ExternalInput")
    wvt_d = nc.dram_tensor("wvt", [C, C], bf16, kind="ExternalInput")
    bqk_d = nc.dram_tensor("bqk", [P, 2 * NK], f32, kind="ExternalInput")
    brow_d = nc.dram_tensor("brow", [1, C + P], bf16, kind="ExternalInput")
    mask_d = nc.dram_tensor("mask", [P, 4 * 512], bf16, kind="ExternalInput")
    bvf_d = nc.dram_tensor("bvfull", [P, C], bf16, kind="ExternalInput")
    out_d = nc.dram_tensor("out", [T, 2 * C], f32, kind="ExternalOutput")

    with tile.TileContext(nc) as tc:
        with (
            tc.tile_pool(name="const", bufs=1) as cpool,
            tc.tile_pool(name="w", bufs=1) as wpool,
            tc.tile_pool(name="xt", bufs=1) as xtpool,
            tc.tile_pool(name="qt", bufs=1) as qtpool,
            tc.tile_pool(name="kt", bufs=1) as ktpool,
            tc.tile_pool(name="v", bufs=1) as vpool,
            tc.tile_pool(name="vp", bufs=1) as vppool,
            tc.tile_pool(name="et", bufs=1) as etpool,
            tc.tile_pool(name="small", bufs=8) as spool,
            tc.tile_pool(name="ostage", bufs=4) as ospool,
            tc.tile_pool(name="psq", bufs=3, space="PSUM") as psq,
            tc.tile_pool(name="psl", bufs=3, space="PSUM") as psl,
            tc.tile_pool(name="pso", bufs=2, space="PSUM") as pso,
        ):
            # --- loads, in first-use order (single HWDGE queue is FIFO) ---
            brow_t = cpool.tile([1, C + P], bf16, name="brow_t")
            nc.sync.dma_start(brow_t[:1, :], brow_d[:1, :])
            bv_row = brow_t[0:1, 0:C]        # [1, 512] bias row for V
            ones_row = brow_t[0:1, C:C + P]  # [1, 128] of ones
            bqk_t = cpool.tile([P, 2 * NK], f32, name="bqk_t")
            nc.sync.dma_start(bqk_t[:], bqk_d[:])

            # PE warm-up: junk matmuls with NO DMA dependency (source is
            # memset on-chip) so they start right after the NEFF prologue.
            # Full-width (Kc=128): the HAM activity monitor meters PE-cell
            # activity and rank-1 matmuls never trip it.  14 matmuls bridge
            # the load window with the clock gate at 8/8 from ~3.4us on,
            # ending about when the first weight/activation tiles land.
            warm_src = cpool.tile([P, C + P], bf16, name="warm_src")
            nc.vector.memset(warm_src[:], 0.0)
            ps_warm = psq.tile([P, 512], f32, name="ps_warm", tag="psq")
            for _ in range(14):
                nc.tensor.matmul(ps_warm[:], warm_src[:, C:C + P],
                                 warm_src[:, 0:C], start=True, stop=True)

            wq_t = []
            for ct in range(NK):
                t_ = wpool.tile([P, C], bf16, name=f"wq{ct}", tag=f"wq{ct}")
                nc.sync.dma_start(t_[:], wqt_d[ct * P:(ct + 1) * P, :])
                wq_t.append(t_)
            xt_t = [xtpool.tile([P, T], bf16, name=f"xt{ct}", tag=f"xt{ct}")
                    for ct in range(NK)]
            for ct in range(NK):  # first QK chunk's worth of X^T
                nc.sync.dma_start(xt_t[ct][:, 0:512], xt_d[ct * P:(ct + 1) * P, 0:512])
            wk_t = []
            for ct in range(NK):
                t_ = wpool.tile([P, C], bf16, name=f"wk{ct}", tag=f"wk{ct}")
                nc.sync.dma_start(t_[:], wkt_d[ct * P:(ct + 1) * P, :])
                wk_t.append(t_)
            for jc in range(1, NJ):  # rest of X^T
                for ct in range(NK):
                    nc.sync.dma_start(xt_t[ct][:, jc * 512:(jc + 1) * 512],
                                      xt_d[ct * P:(ct + 1) * P, jc * 512:(jc + 1) * 512])
            wv_t = []
            for ct in range(NK):
                t_ = wpool.tile([P, C], bf16, name=f"wv{ct}", tag=f"wv{ct}")
                nc.sync.dma_start(t_[:], wvt_d[ct * P:(ct + 1) * P, :])
                wv_t.append(t_)
            bvf_t = cpool.tile([P, C], bf16, name="bvf_t")
            nc.sync.dma_start(bvf_t[:], bvf_d[:])
            mask_t = cpool.tile([P, 4 * 512], bf16, name="mask_t")
            nc.sync.dma_start(mask_t[:], mask_d[:])

            # --- Phase 1a: Q^T, K^T in [k, t] layout ---
            # Q^T[k, t] = sum_c WqT[c, k] * XT[c, t]  (+ bias per partition).
            qt_t = [qtpool.tile([P, T], bf16, name=f"qt{k}", tag=f"qt{k}")
                    for k in range(NK)]
            kt_t = [ktpool.tile([P, T], bf16, name=f"kt{k}", tag=f"kt{k}")
                    for k in range(NK)]
            for jc in range(NJ):
                js = slice(jc * 512, (jc + 1) * 512)
                for kt in range(NK):
                    ksl = slice(kt * P, (kt + 1) * P)
                    ps = psq.tile([P, 512], f32, name="psq1", tag="psq")
                    for ct in range(NK):
                        nc.tensor.matmul(ps[:], wq_t[ct][:, ksl],
                                         xt_t[ct][:, js],
                                         start=(ct == 0), stop=(ct == NK - 1))
                    nc.vector.tensor_scalar_add(qt_t[kt][:, js], ps[:],
                                                bqk_t[:, kt:kt + 1])
                    ps2 = psq.tile([P, 512], f32, name="psq2", tag="psq")
                    for ct in range(NK):
                        nc.tensor.matmul(ps2[:], wk_t[ct][:, ksl],
                                         xt_t[ct][:, js],
                                         start=(ct == 0), stop=(ct == NK - 1))
                    nc.vector.tensor_scalar_add(kt_t[kt][:, js], ps2[:],
                                                bqk_t[:, NK + kt:NK + kt + 1])

            # --- Phase 1b: V in natural [t, v] layout ---
            # Bias added on DVE from a pre-broadcast [128, 512] tile (the
            # bias is per free-dim column, so no per-partition trick applies
            # and the DVE rejects zero-stride partition APs).
            v_t = []
            for tt in range(NT):
                ps = psq.tile([P, 512], f32, name="psv", tag="psq")
                for ct in range(NK):
                    nc.tensor.matmul(ps[:], xt_t[ct][:, tt * P:(tt + 1) * P],
                                     wv_t[ct][:],
                                     start=(ct == 0), stop=(ct == NK - 1))
                vt = vpool.tile([P, 512], bf16, name=f"v{tt}", tag=f"v{tt}")
                nc.vector.tensor_add(vt[:], ps[:], bvf_t[:])
                v_t.append(vt)

            # --- Phase 2: masked logits + exp + row sums, per key tile ---
            et_t = [etpool.tile([P, T], bf16, name=f"et{i}", tag=f"et{i}")
                    for i in range(NT)]
            vp_t = []
            for it in range(NT):
                jc0 = it // 4
                m = it % 4
                isl = slice(it * P, (it + 1) * P)
                parts = []
                for jc in range(jc0, NJ):
                    # Trim the leading fully-masked 128-col strips of the
                    # diagonal chunk (cols with j < 128*it for every row).
                    off = 128 * m if jc == jc0 else 0
                    w = 512 - off
                    js = slice(jc * 512 + off, (jc + 1) * 512)
                    ps = psl.tile([P, 512], f32, name="psl", tag="psl")
                    for kt in range(NK):
                        nc.tensor.matmul(ps[:, 0:w], kt_t[kt][:, isl],
                                         qt_t[kt][:, js],
                                         start=(kt == 0), stop=(kt == NK - 1))
                    if jc == jc0:
                        nc.vector.tensor_add(ps[:, 0:w], ps[:, 0:w],
                                             mask_t[:, m * 512 + off:(m + 1) * 512])
                    part = spool.tile([P, 1], f32, name="part", tag="part")
                    nc.scalar.activation(et_t[it][:, js], ps[:, 0:w], AF.Exp,
                                         accum_out=part[:])
                    parts.append(part)
                if len(parts) == 1:
                    s = parts[0]
                else:
                    s = spool.tile([P, 1], f32, name="s", tag="s")
                    nc.vector.tensor_add(s[:], parts[0][:], parts[1][:])
                    for p_ in parts[2:]:
                        nc.vector.tensor_add(s[:], s[:], p_[:])
                r = spool.tile([P, 1], f32, name="r", tag="r")
                nc.vector.reciprocal(r[:], s[:])
                vp = vppool.tile([P, 512], bf16, name=f"vp{it}", tag=f"vp{it}")
                nc.vector.tensor_scalar_mul(vp[:], v_t[it][:], r[:])
                vp_t.append(vp)

            # --- Phase 3: read[jt] = sum_{it<=jt} E^T[it][:, jt].T @ V'[it] ---
            # The last two rows split their accumulation into two chains so
            # the post-phase-2 critical path is ~8 matmuls, not 16.
            for jt in range(NT):
                jsl = slice(jt * P, (jt + 1) * P)
                ost = ospool.tile([P, 512], f32, name="ost", tag="ost")
                if jt >= NT - 2:
                    ha = (jt + 1) // 2
                    psa = psq.tile([P, 512], f32, name="psa", tag="psq")
                    for it in range(ha):
                        nc.tensor.matmul(psa[:], et_t[it][:, jsl], vp_t[it][:],
                                         start=(it == 0), stop=(it == ha - 1))
                    # stage the early half in SBUF (off the critical tail;
                    # also the DVE cannot read two PSUM operands at once)
                    sba = ospool.tile([P, 512], f32, name="sba", tag="sba")
                    nc.vector.tensor_copy(sba[:], psa[:])
                    psb = pso.tile([P, 512], f32, name="psb", tag="pso")
                    for it in range(ha, jt + 1):
                        nc.tensor.matmul(psb[:], et_t[it][:, jsl], vp_t[it][:],
                                         start=(it == ha), stop=(it == jt))
                    nc.vector.tensor_add(ost[:], sba[:], psb[:])
                else:
                    ps = pso.tile([P, 512], f32, name="pso", tag="pso")
                    for it in range(jt + 1):
                        nc.tensor.matmul(ps[:], et_t[it][:, jsl], vp_t[it][:],
                                         start=(it == 0), stop=(it == jt))
                    nc.vector.tensor_copy(ost[:], ps[:])
                nc.gpsimd.dma_start(out_d[jsl, C:2 * C], ost[:])

            # --- Input passthrough: out[:, 0:512] = x ---
            # On the same HWDGE queue as the loads, emitted last: the FIFO
            # keeps it behind every load so it cannot starve them, and it
            # still finishes well inside the compute window.
            for i in range(4):
                r0 = i * (T // 4)
                nc.sync.dma_start(out_d[r0:r0 + T // 4, 0:C],
                                  x_d[r0:r0 + T // 4, :])

    nc.compile()
    return nc


def _get_built():
    global _BUILT
    if _BUILT is None:
        _BUILT = _build_nc()
    return _BUILT


def _make_in_maps(input, Wq, bq, Wk, bk, Wv, bv):
    bf = ml_dtypes.bfloat16
    s = 1.0 / math.sqrt(C)

    input = np.asarray(input, np.float32)
    Wq = np.asarray(Wq, np.float32)
    bq = np.asarray(bq, np.float32)
    Wk = np.asarray(Wk, np.float32)
    bk = np.asarray(bk, np.float32)
    Wv = np.asarray(Wv, np.float32)
    bv = np.asarray(bv, np.float32)

    # 1/sqrt(512) folded into the query projection.
    wqt = np.ascontiguousarray((Wq * s).T).astype(bf)
    wkt = np.ascontiguousarray(Wk.T).astype(bf)
    wvt = np.ascontiguousarray(Wv.T).astype(bf)

    bqk = np.empty((P, 2 * NK), np.float32)
    for kt in range(NK):
        bqk[:, kt] = bq[kt * P:(kt + 1) * P] * s
        bqk[:, NK + kt] = bk[kt * P:(kt + 1) * P]

    brow = np.empty((1, C + P), bf)
    brow[0, :C] = bv.astype(bf)
    brow[0, C:] = np.float32(1.0)
    bvfull = np.ascontiguousarray(np.broadcast_to(bv.astype(bf), (P, C)))

    # mask m: row p (key i = 128*it + p), col x (query j = 512*(it//4) + x):
    # masked (j < i) iff x < p + 128*m where m = it % 4.
    pp = np.arange(P)[:, None]
    xx = np.arange(512)[None, :]
    mask = np.empty((P, 4 * 512), np.float32)
    for m in range(4):
        mask[:, m * 512:(m + 1) * 512] = np.where(xx < pp + 128 * m, NEG, 0.0)
    mask = mask.astype(bf)

    in_maps = []
    for b in range(B):
        xb = np.ascontiguousarray(input[b])
        in_maps.append({
            "xt": np.ascontiguousarray(xb.T).astype(bf),
            "x": xb,
            "wqt": wqt, "wkt": wkt, "wvt": wvt,
            "bqk": bqk, "brow": brow, "mask": mask, "bvfull": bvfull,
        })
    return in_maps


def kernel(input, Wq, bq, Wk, bk, Wv, bv, _trace=False):
    from concourse.bass_utils import run_bass_kernel_spmd

    nc = _get_built()
    in_maps = _make_in_maps(input, Wq, bq, Wk, bk, Wv, bv)
    res = run_bass_kernel_spmd(nc, in_maps, core_ids=list(range(NCORES)),
                               trace=_trace)
    out = np.stack([r["out"] for r in res.results], axis=0)
    if _trace:
        kernel.last_result = res
    return out

